# revision 8
# baseline (speedup 1.0000x reference)
"""Trainium2 Bass kernel for the dual-attention module (spatial + channel attention).

Contract: kernel(**inputs) takes the FULL inputs (x: (16,1024,64,64) f32 plus four
1x1-conv weight matrices) and returns the FULL output (16,1024,64,64) f32.
Internally shards data-parallel over batch across 8 NeuronCores (2 samples/core),
weights replicated.

Per-sample math (b, c=1024, ch=512, hw=4096):
  conv(w) = relu(w @ X)               X = x[b] as (1024, 4096)
  mask    = softmax(conv(w_qr))       over hw          (spatial attn branch)
  ctx     = conv(w_vr) @ mask         (ch,)
  s       = sigmoid(layernorm(ctx))   (ch,)
  avg     = softmax(mean_hw(conv(w_ql)))               (channel attn branch)
  chan    = sigmoid(avg @ conv(w_vl)) (hw,)
  out[0:512]    = x * (1 + s*chan)                     ("sequence")
  out[512:1024] = x * (1 + s + chan)                   ("parallel")

The PE streams rhs at 1 row/cycle, so every N=512 matmul costs ~235ns regardless
of dtype; fp8 DoubleRow packs 2 k-tiles per instruction, halving instruction
count vs fp32r.  All four convs therefore run in fp8e4m3 DoubleRow (weights
pre-scaled x64 into fp8 range; the 1/64 is folded into later scalar passes or is
invariant under layer_norm).  Quantization noise reaches the output only through
softmax/mean averages over 4096 (or 512) terms plus sigmoids; measured max rel
err vs the f32 reference is ~1.5e-2 against the 2e-2 gate.

Schedule (per core; hw column dim split into 4 chunk-PAIRS of 2x512):
  - sample-0 phase A (qr+vr convs), finalize; phase B (ql conv), finalize;
    then sample-0 phase C (vl conv + chan attn + finale) with sample-1 phase A
    interleaved pair-by-pair so the PE never drains while finale engines work;
    then sample-1 B and C.
  - Finale elementwise ops run PAIR-wide ([128, 1024]) to amortize DVE/GpSimd
    instruction overhead: 4 "parallel" tiles as single fused DVE
    scalar_tensor_tensor passes; 4 "sequence" tiles as attn-construction
    (2 ACT copies w/ scale+bias, 2 GpSimd tensor_scalar) + multiply (2 DVE,
    2 GpSimd).  This keeps every engine under the PE's pace.
  - xq (fp8 image) is the matmul operand, resident per sample; x arrives as
    bf16 purely for the finale multiply.  All DMA goes through the Sync DGE;
    next-sample prefetches are emitted where their pool buffer is already
    free so a waiting trigger never head-of-line-blocks the queue.
  - Output f32 accumulates into pair SBUF tiles [128, 2, 512]; stores are
    whole-tile DMAs (8 per pair; the last pair is split per-chunk to halve
    the end-of-kernel store burst).
  - Softmaxes are computed unnormalized (exp only); 1/Z folded into later
    scalars.  exp(relu(z)) == max(exp(z),1).  qr conv uses column-replicated
    weights so psum rows arrive broadcast; the chan contraction uses
    replicated e-weights (fp32r).  Cross-partition reductions via exact-f32
    gpsimd.partition_all_reduce.
"""

import sys

sys.path.insert(0, "/opt/trn_rl_repo")

import numpy as np

import concourse.bass as bass  # noqa: F401  (bass must import before bacc)
import concourse.tile as tile
from concourse import bacc, bass_isa, bass_utils, mybir

# Problem constants (hardcoded per contract).
B, C, H, W = 16, 1024, 64, 64
HW = H * W               # 4096
CH = C // 2              # 512
N_CORES = 8
S = B // N_CORES         # 2 samples per core
P = 128                  # SBUF partitions
KT = C // P              # 8 k-tiles over input channels
A2 = KT // 2             # 4 DoubleRow k-pair steps
MT = CH // P             # 4 m-tiles over output channels
NW = 512                 # n-chunk width (one PSUM bank of f32)
NCH = HW // NW           # 8 n-chunks
NP = NCH // 2            # 4 chunk-pairs
LN_EPS = 1e-5
WS = 64.0                # fp8 weight pre-scale

F32 = mybir.dt.float32
F32R = mybir.dt.float32r
BF16 = mybir.dt.bfloat16
F8 = mybir.dt.float8e4
Alu = mybir.AluOpType
Act = mybir.ActivationFunctionType
AxX = mybir.AxisListType.X
DR = mybir.MatmulPerfMode.DoubleRow

_cache = {}


def _build():
    nc = bacc.Bacc(
        "TRN2",
        target_bir_lowering=False,
        debug=False,
        num_devices=N_CORES,
        dynamic_dma_scratch_size=512,
    )

    # pair-major layouts: one pair is a single DMA with contiguous bytes per
    # partition; weights partition-major.
    xq_d = nc.dram_tensor("xq", [S, NP, P, KT, 2, NW], F8, kind="ExternalInput")
    x_d = nc.dram_tensor("x", [S, NP, P, KT, 2, NW], BF16, kind="ExternalInput")
    wqr_d = nc.dram_tensor("wqr", [P, KT, P], F8, kind="ExternalInput")
    wvr_d = nc.dram_tensor("wvr", [P, KT, CH], F8, kind="ExternalInput")
    wql_d = nc.dram_tensor("wql", [P, KT, CH], F8, kind="ExternalInput")
    wvl_d = nc.dram_tensor("wvl", [P, KT, CH], F8, kind="ExternalInput")
    out_d = nc.dram_tensor("out", [S, KT, P, NP, 2, NW], F32, kind="ExternalOutput")

    with tile.TileContext(nc) as tc:
        with (
            tc.tile_pool(name="xqp", bufs=NP + 1) as xqp,
            tc.tile_pool(name="xp", bufs=NP) as xp,
            tc.tile_pool(name="wp", bufs=1) as wp,
            tc.tile_pool(name="okp", bufs=1) as okp,
            tc.tile_pool(name="actp", bufs=2) as actp,
            tc.tile_pool(name="deadp", bufs=1) as deadp,
            tc.tile_pool(name="thp", bufs=5) as thp,
            tc.tile_pool(name="atp", bufs=5) as atp,
            tc.tile_pool(name="smp", bufs=2) as smp,
            tc.tile_pool(name="erp", bufs=2 * MT) as erp,
            tc.tile_pool(name="psA", bufs=3, space="PSUM") as psA,
            tc.tile_pool(name="psB", bufs=5, space="PSUM") as psB,
        ):
            # ---- constants ----
            epst = wp.tile([P, 1], F32, name="epst", tag="epst")
            nc.vector.memset(epst[:], LN_EPS)

            # ---- weight tiles ----
            wqr_sb = wp.tile([P, KT, P], F8, name="wqrsb", tag="wqrsb")
            wvr_sb = wp.tile([P, KT, CH], F8, name="wvrsb", tag="wvrsb")
            wql_sb = wp.tile([P, KT, CH], F8, name="wqlsb", tag="wqlsb")
            wvl_sb = wp.tile([P, KT, CH], F8, name="wvlsb", tag="wvlsb")
            wdma = {"wqr": wqr_d, "wvr": wvr_d, "wql": wql_d, "wvl": wvl_d}

            def load_w(t, nm):
                nc.sync.dma_start(t[:], wdma[nm].ap()[:])

            def emit_xq_load(s_, j_):
                t = xqp.tile([P, KT, 2, NW], F8, name=f"xq{s_}_{j_}", tag="xq")
                nc.sync.dma_start(t[:], xq_d.ap()[s_, j_])
                return t

            def emit_x_load(s_, j_):
                t = xp.tile([P, KT, 2, NW], BF16, name=f"x{s_}_{j_}", tag="x")
                nc.sync.dma_start(t[:], x_d.ap()[s_, j_])
                return t

            # sample-0 loads: the first matmul gates on exactly two triggers
            # (wqr + xq pair 0); everything else queues behind.  bf16 x
            # (finale-only) trails.
            xq_all = {0: [], 1: []}
            x_all = {0: [], 1: []}
            load_w(wqr_sb, "wqr")
            xq_all[0].append(emit_xq_load(0, 0))
            load_w(wvr_sb, "wvr")
            xq_all[0].append(emit_xq_load(0, 1))
            load_w(wql_sb, "wql")
            xq_all[0].append(emit_xq_load(0, 2))
            load_w(wvl_sb, "wvl")
            xq_all[0].append(emit_xq_load(0, 3))
            for j in range(NP):
                x_all[0].append(emit_x_load(0, j))

            # per-sample state, created lazily at first phase-A emission
            st = {}

            def mk_state(s):
                st[s] = d = {}
                d["zpart"] = smp.tile([P, NCH], F32, name=f"zpart{s}", tag="zpart")
                d["ctxp"] = [
                    smp.tile([P, NCH], F32, name=f"ctxp{s}_{m}", tag=f"ctxp{m}")
                    for m in range(MT)
                ]
                d["gp"] = [
                    smp.tile([P, NCH], F32, name=f"gp{s}_{m}", tag=f"gp{m}")
                    for m in range(MT)
                ]

            def phaseA_chunk(s, j, q):
                """qr conv (mask logits) + vr conv (context) for chunk 2j+q."""
                d = st[s]
                i = 2 * j + q
                xqj = xq_all[s][j]
                rhs = xqj[:, :, q, :]
                psq = psA.tile([P, NW], F32, name=f"psq{s}_{i}", tag="psA")
                for a in range(A2):
                    nc.tensor.matmul(
                        psq[:],
                        wqr_sb[:, 2 * a : 2 * a + 2, :],
                        rhs[:, 2 * a : 2 * a + 2, :],
                        start=(a == 0), stop=(a == A2 - 1),
                        perf_mode=DR,
                    )
                # exp(relu(z)) == max(exp(z), 1): ACT exp (1/64 de-scales the
                # fp8 weight prescale), then DVE in-place max + Z accum
                et = actp.tile([P, NW], F32, name=f"et{s}_{i}", tag="et", bufs=3)
                nc.scalar.activation(et[:], psq[:], Act.Exp, scale=1.0 / WS)
                nc.vector.tensor_scalar(
                    et[:], et[:], 1.0, 0.0, Alu.max, Alu.add,
                    accum_out=d["zpart"][:, i : i + 1],
                )
                for m in range(MT):
                    psv = psB.tile([P, NW], F32, name=f"psv{s}a{i}_{m}", tag="psB")
                    for a in range(A2):
                        nc.tensor.matmul(
                            psv[:],
                            wvr_sb[:, 2 * a : 2 * a + 2, m * P : (m + 1) * P],
                            rhs[:, 2 * a : 2 * a + 2, :],
                            start=(a == 0), stop=(a == A2 - 1),
                            perf_mode=DR,
                        )
                    # ctx partial: sum_n relu(vr) * exp(relu(qr))
                    scr = deadp.tile([P, NW], F32, name=f"sttscr{s}", tag="sttscr")
                    nc.vector.scalar_tensor_tensor(
                        scr[:], psv[:], 0.0, et[:], Alu.max, Alu.mult,
                        accum_out=d["ctxp"][m][:, i : i + 1],
                    )

            def finalizeA(s):
                """mask Z + context -> layernorm stats."""
                d = st[s]
                Zt = smp.tile([P, 1], F32, name=f"Z{s}", tag="Z")
                nc.vector.tensor_reduce(Zt[:], d["zpart"][:], AxX, Alu.add)
                rZ = smp.tile([P, 1], F32, name=f"rZ{s}", tag="rZ")
                nc.vector.reciprocal(rZ[:], Zt[:])
                ctx44 = smp.tile([P, MT], F32, name=f"ctx44{s}", tag="ctx44")
                for m in range(MT):
                    cred = smp.tile([P, 1], F32, name=f"cred{s}_{m}", tag="cred")
                    nc.vector.tensor_reduce(cred[:], d["ctxp"][m][:], AxX, Alu.add)
                    # 1/64 restores the fp8 prescale: reference LN eps semantics
                    nc.vector.tensor_scalar(
                        ctx44[:, m : m + 1], cred[:], rZ[:], 1.0 / WS,
                        Alu.mult, Alu.mult,
                    )
                lnsum = smp.tile([P, MT], F32, name=f"lnsum{s}", tag="lnsum")
                nc.gpsimd.partition_all_reduce(
                    lnsum[:], ctx44[:], P, bass_isa.ReduceOp.add
                )
                tot = smp.tile([P, 1], F32, name=f"tot{s}", tag="tot")
                nc.vector.tensor_reduce(tot[:], lnsum[:], AxX, Alu.add)
                mu = smp.tile([P, 1], F32, name=f"mu{s}", tag="mu")
                nc.vector.tensor_scalar(mu[:], tot[:], 1.0 / CH, None, Alu.mult)
                d44 = smp.tile([P, MT], F32, name=f"d44{s}", tag="d44")
                nc.vector.tensor_scalar(d44[:], ctx44[:], mu[:], None, Alu.subtract)
                d2 = smp.tile([P, MT], F32, name=f"d2{s}", tag="d2")
                nc.vector.tensor_tensor(d2[:], d44[:], d44[:], Alu.mult)
                vsum = smp.tile([P, MT], F32, name=f"vsum{s}", tag="vsum")
                nc.gpsimd.partition_all_reduce(
                    vsum[:], d2[:], P, bass_isa.ReduceOp.add
                )
                vtot = smp.tile([P, 1], F32, name=f"vtot{s}", tag="vtot")
                nc.vector.tensor_reduce(vtot[:], vsum[:], AxX, Alu.add)
                var = smp.tile([P, 1], F32, name=f"var{s}", tag="var")
                nc.vector.tensor_scalar(var[:], vtot[:], 1.0 / CH, None, Alu.mult)
                d["ctx44"], d["mu"], d["var"] = ctx44, mu, var

            def phaseB_chunk(s, j, q):
                """ql conv chunk; relu + mean partials, alternating engines."""
                d = st[s]
                i = 2 * j + q
                rhs = xq_all[s][j][:, :, q, :]
                for m in range(MT):
                    psv = psB.tile([P, NW], F32, name=f"psv{s}b{i}_{m}", tag="psB")
                    for a in range(A2):
                        nc.tensor.matmul(
                            psv[:],
                            wql_sb[:, 2 * a : 2 * a + 2, m * P : (m + 1) * P],
                            rhs[:, 2 * a : 2 * a + 2, :],
                            start=(a == 0), stop=(a == A2 - 1),
                            perf_mode=DR,
                        )
                    if m % 2 == 0:
                        scr = deadp.tile([P, NW], F32, name=f"qlscr{s}", tag="qlscr")
                        nc.scalar.activation(
                            scr[:], psv[:], Act.Relu,
                            accum_out=d["gp"][m][:, i : i + 1],
                        )
                    else:
                        scr2 = deadp.tile([P, NW], F32, name=f"sttscr{s}b", tag="sttscr")
                        nc.vector.tensor_scalar(
                            scr2[:], psv[:], 0.0, 0.0, Alu.max, Alu.add,
                            accum_out=d["gp"][m][:, i : i + 1],
                        )

            def finalizeB(s):
                """chan-softmax weights e, LN sigmoid -> s44/sp44, erep."""
                d = st[s]
                g44 = smp.tile([P, MT], F32, name=f"g44{s}", tag="g44")
                for m in range(MT):
                    nc.vector.tensor_reduce(
                        g44[:, m : m + 1], d["gp"][m][:], AxX, Alu.add
                    )
                e44 = smp.tile([P, MT], F32, name=f"e44{s}", tag="e44")
                nc.scalar.activation(e44[:], g44[:], Act.Exp, scale=1.0 / (HW * WS))
                std = smp.tile([P, 1], F32, name=f"std{s}", tag="std")
                nc.scalar.activation(std[:], d["var"][:], Act.Sqrt, bias=epst[:])
                rstd = smp.tile([P, 1], F32, name=f"rstd{s}", tag="rstd")
                nc.vector.reciprocal(rstd[:], std[:])
                spre = smp.tile([P, MT], F32, name=f"spre{s}", tag="spre")
                nc.vector.tensor_scalar(
                    spre[:], d["ctx44"][:], d["mu"][:], rstd[:],
                    Alu.subtract, Alu.mult,
                )
                s44 = smp.tile([P, MT], F32, name=f"s44{s}", tag="s44")
                nc.scalar.activation(s44[:], spre[:], Act.Sigmoid)
                sp44 = smp.tile([P, MT], F32, name=f"sp44{s}", tag="sp44")
                nc.vector.tensor_scalar(sp44[:], s44[:], 1.0, None, Alu.add)
                ze = smp.tile([P, MT], F32, name=f"ze{s}", tag="ze")
                nc.gpsimd.partition_all_reduce(ze[:], e44[:], P, bass_isa.ReduceOp.add)
                zet = smp.tile([P, 1], F32, name=f"zet{s}", tag="zet")
                nc.vector.tensor_reduce(zet[:], ze[:], AxX, Alu.add)
                rZc = smp.tile([P, 1], F32, name=f"rZc{s}", tag="rZc")
                nc.vector.reciprocal(rZc[:], zet[:])
                erep = []
                for m in range(MT):
                    er = erp.tile([P, P], F32R, name=f"erep{s}_{m}", tag="erep")
                    # 1/64 compensates the x64 fp8 scaling of wvl
                    nc.vector.tensor_scalar(
                        er[:], e44[:, m : m + 1].broadcast_to([P, P]),
                        1.0 / WS, None, Alu.mult,
                    )
                    erep.append(er)
                d["s44"], d["sp44"], d["rZc"], d["erep"] = s44, sp44, rZc, erep

            def phaseC_pair_matmuls(s, j):
                """vl conv + chan contraction + sigmoid for both chunks of
                pair j; returns the pair-wide chan tile."""
                d = st[s]
                chant = actp.tile(
                    [P, 2, NW], F32, name=f"ch{s}_{j}", tag="chant", bufs=2
                )
                for q in range(2):
                    i = 2 * j + q
                    rhs = xq_all[s][j][:, :, q, :]
                    thl = []
                    for m in range(MT):
                        psv = psB.tile([P, NW], F32, name=f"psv{s}c{i}_{m}", tag="psB")
                        for a in range(A2):
                            nc.tensor.matmul(
                                psv[:],
                                wvl_sb[:, 2 * a : 2 * a + 2, m * P : (m + 1) * P],
                                rhs[:, 2 * a : 2 * a + 2, :],
                                start=(a == 0), stop=(a == A2 - 1),
                                perf_mode=DR,
                            )
                        th = thp.tile([P, NW], F32R, name=f"th{s}_{i}_{m}", tag="th")
                        nc.scalar.activation(th[:], psv[:], Act.Relu)
                        thl.append(th)
                    pschan = psA.tile([P, NW], F32, name=f"psc{s}_{i}", tag="psA")
                    for m in range(MT):
                        nc.tensor.matmul(
                            pschan[:], d["erep"][m][:], thl[m][:],
                            start=(m == 0), stop=(m == MT - 1),
                            skip_group_check=True,
                        )
                    nc.scalar.activation(
                        chant[:, q, :], pschan[:], Act.Sigmoid, scale=d["rZc"][:]
                    )
                return chant

            def phaseC_pair_finale(s, j, chant):
                """pair-wide finale: seq rows k<4: x*(1 + s*chan); par rows
                k>=4: x*(chan+1+s); then stores."""
                d = st[s]
                s44, sp44 = d["s44"], d["sp44"]
                xpair = x_all[s][j]
                okq = []
                for k in range(KT):
                    okq.append(
                        okp.tile([P, 2, NW], F32, name=f"ok{s}_{j}_{k}", tag=f"ok{k}")
                    )
                for k in range(MT, KT):
                    nc.vector.scalar_tensor_tensor(
                        okq[k][:], chant[:], sp44[:, k - MT : k - MT + 1],
                        xpair[:, k, :, :], Alu.add, Alu.mult,
                    )
                at = []
                for k in range(2):
                    a_t = atp.tile([P, 2, NW], F32, name=f"at{s}_{j}_{k}", tag="at")
                    nc.scalar.activation(
                        a_t[:], chant[:], Act.Copy,
                        scale=s44[:, k : k + 1], bias=1.0,
                    )
                    at.append(a_t)
                for k in range(2, MT):
                    a_t = atp.tile([P, 2, NW], F32, name=f"at{s}_{j}_{k}", tag="at")
                    nc.gpsimd.tensor_scalar(
                        a_t[:], chant[:], s44[:, k : k + 1], 1.0,
                        Alu.mult, Alu.add,
                    )
                    at.append(a_t)
                for k in range(2):
                    nc.vector.tensor_tensor(
                        okq[k][:], at[k][:], xpair[:, k, :, :], Alu.mult
                    )
                for k in range(2, MT):
                    nc.gpsimd.tensor_tensor(
                        okq[k][:], at[k][:], xpair[:, k, :, :], Alu.mult
                    )
                for k in range(KT):
                    nc.sync.dma_start(out_d.ap()[s, k, :, j], okq[k][:])

            def phaseC_last_pair(s, j):
                """final pair, chunk-at-a-time: chunk q=0's finale and stores
                are emitted before chunk q=1's matmuls so they drain under the
                PE; after the last matmul only one chunk of finale remains."""
                d = st[s]
                s44, sp44 = d["s44"], d["sp44"]
                xpair = x_all[s][j]
                for q in range(2):
                    i = 2 * j + q
                    rhs = xq_all[s][j][:, :, q, :]
                    thl = []
                    for m in range(MT):
                        psv = psB.tile([P, NW], F32, name=f"psv{s}c{i}_{m}", tag="psB")
                        for a in range(A2):
                            nc.tensor.matmul(
                                psv[:],
                                wvl_sb[:, 2 * a : 2 * a + 2, m * P : (m + 1) * P],
                                rhs[:, 2 * a : 2 * a + 2, :],
                                start=(a == 0), stop=(a == A2 - 1),
                                perf_mode=DR,
                            )
                        th = thp.tile([P, NW], F32R, name=f"th{s}_{i}_{m}", tag="th")
                        nc.scalar.activation(th[:], psv[:], Act.Relu)
                        thl.append(th)
                    pschan = psA.tile([P, NW], F32, name=f"psc{s}_{i}", tag="psA")
                    for m in range(MT):
                        nc.tensor.matmul(
                            pschan[:], d["erep"][m][:], thl[m][:],
                            start=(m == 0), stop=(m == MT - 1),
                            skip_group_check=True,
                        )
                    chant = actp.tile(
                        [P, NW], F32, name=f"chq{s}_{i}", tag="chant", bufs=2
                    )
                    nc.scalar.activation(
                        chant[:], pschan[:], Act.Sigmoid, scale=d["rZc"][:]
                    )
                    okq = []
                    for k in range(KT):
                        okq.append(
                            okp.tile([P, NW], F32, name=f"okl{s}_{i}_{k}", tag=f"okl{k}")
                        )
                    for k in range(MT, KT):
                        nc.vector.scalar_tensor_tensor(
                            okq[k][:], chant[:], sp44[:, k - MT : k - MT + 1],
                            xpair[:, k, q, :], Alu.add, Alu.mult,
                        )
                    at = []
                    for k in range(2):
                        a_t = atp.tile([P, NW], F32, name=f"atl{s}_{i}_{k}", tag="at")
                        nc.scalar.activation(
                            a_t[:], chant[:], Act.Copy,
                            scale=s44[:, k : k + 1], bias=1.0,
                        )
                        at.append(a_t)
                    for k in range(2, MT):
                        a_t = atp.tile([P, NW], F32, name=f"atl{s}_{i}_{k}", tag="at")
                        nc.gpsimd.tensor_scalar(
                            a_t[:], chant[:], s44[:, k : k + 1], 1.0,
                            Alu.mult, Alu.add,
                        )
                        at.append(a_t)
                    for k in range(2):
                        nc.vector.tensor_tensor(
                            okq[k][:], at[k][:], xpair[:, k, q, :], Alu.mult
                        )
                    for k in range(2, MT):
                        nc.gpsimd.tensor_tensor(
                            okq[k][:], at[k][:], xpair[:, k, q, :], Alu.mult
                        )
                    for k in range(KT):
                        nc.sync.dma_start(
                            out_d.ap()[s, k, :, j, q, :], okq[k][:]
                        )

            # ================= schedule =================
            mk_state(0)
            for j in range(NP):
                for q in range(2):
                    phaseA_chunk(0, j, q)
            finalizeA(0)
            for j in range(NP):
                for q in range(2):
                    phaseB_chunk(0, j, q)
            finalizeB(0)
            # sample-0 phase C with sample-1 phases A AND B interleaved
            # pair-by-pair: the PE does ~112 instructions per iteration while
            # the finale engines (DVE/ACT/GpSimd, ~19us) run far under its
            # ~26us pace, so sample-1's phase C starts with no backlog.
            mk_state(1)
            for j in range(NP):
                # next sample's xq pair: buffer already free (pool keeps one
                # spare); DMA runs under this pair's matmuls
                xq_all[1].append(emit_xq_load(1, j))
                chant = phaseC_pair_matmuls(0, j)
                phaseA_chunk(1, j, 0)
                phaseA_chunk(1, j, 1)
                phaseB_chunk(1, j, 0)
                phaseB_chunk(1, j, 1)
                phaseC_pair_finale(0, j, chant)
                # next sample's bf16 x pair: buffer freed by the finale above
                x_all[1].append(emit_x_load(1, j))
            finalizeA(1)
            finalizeB(1)
            for j in range(NP - 1):
                chant = phaseC_pair_matmuls(1, j)
                phaseC_pair_finale(1, j, chant)
            # last pair per-chunk so chunk 6's finale + stores drain under
            # chunk 7's matmuls: only one chunk of work follows the last matmul
            phaseC_last_pair(1, NP - 1)

    nc.compile()
    return nc


def _prep_inputs(x, w_qr, w_vr, w_ql, w_vl):
    import ml_dtypes

    f8 = np.dtype(ml_dtypes.float8_e4m3)
    bf16 = np.dtype(ml_dtypes.bfloat16)
    x = np.asarray(x, dtype=np.float32).reshape(B, C, HW)
    wts = {}
    for nm, w in (("wvr", w_vr), ("wql", w_ql), ("wvl", w_vl)):
        w = np.asarray(w, dtype=np.float32)
        # (out, in) -> [P, KT, out]: wts[nm][p, k, o] = w[o, 128k + p]
        # scaled x64 into fp8 range (the 1/64 is folded back on-chip)
        wts[nm] = (
            np.ascontiguousarray(w.T.reshape(KT, P, CH).transpose(1, 0, 2)) * WS
        ).astype(f8)
    q = np.asarray(w_qr, dtype=np.float32).reshape(KT, P).T * WS  # [P, KT]
    wts["wqr"] = np.ascontiguousarray(
        np.broadcast_to(q[:, :, None], (P, KT, P))
    ).astype(f8)
    in_maps = []
    for c in range(N_CORES):
        m = dict(wts)
        # [S, pair, P, KT, 2, NW]: xf[s,j,p,k,q,n] = x[s, 128k+p, 512(2j+q)+n]
        xf = np.ascontiguousarray(
            x[S * c : S * (c + 1)]
            .reshape(S, KT, P, NP, 2, NW)
            .transpose(0, 3, 2, 1, 4, 5)
        )
        m["x"] = xf.astype(bf16)
        m["xq"] = xf.astype(f8)
        in_maps.append(m)
    return in_maps


def _run(x, w_qr, w_vr, w_ql, w_vl, trace=False):
    if "nc" not in _cache:
        _cache["nc"] = _build()
    nc = _cache["nc"]
    in_maps = _prep_inputs(x, w_qr, w_vr, w_ql, w_vl)
    res = bass_utils.run_bass_kernel_spmd(
        nc, in_maps, core_ids=list(range(N_CORES)), trace=trace
    )
    out = np.empty((B, C, HW), np.float32)
    for c in range(N_CORES):
        # [S, KT, P, NP, 2, NW] f32 -> [S, C, HW]
        out[S * c : S * (c + 1)] = res.results[c]["out"].reshape(S, C, HW)
    return out.reshape(B, C, H, W), res


def kernel(x, w_qr, w_vr, w_ql, w_vl):
    out, _ = _run(x, w_qr, w_vr, w_ql, w_vl, trace=False)
    return out
